# revision 9
# baseline (speedup 1.0000x reference)
"""GRU cell kernel for Trainium2, data-parallel over batch across 8 NeuronCores.

Reference computation (B=8192, D=H=1024), per batch row:
    z = sigmoid(inp@wz + state@uz + bz)
    r = sigmoid(inp@wr + state@ur + br)
    h_ = tanh(inp@wx + bx + (state@wh) * r)
    hid = (1-z)*h_ + state*z

Strategy: each core takes a 1024-row batch shard. The z/r projections fuse
into one [1024,2048]@[2048,2048] GEMM (act = [inp|state], W = [[wz,wr],[uz,ur]]).
xh and hh stay separate GEMMs because hh is gated by r before the sum.
All matmul operands are bf16 (fp32 PSUM accumulate): the moving operand
streams one 512-wide column per PE cycle and the bf16 LDWEIGHTS (~97ns)
hides under the previous matmul, so the PE runs at the 213ns/matmul
streaming floor. Weights and transposed activations are retiled on the
host so every DMA lands as a few large contiguous runs per partition
(fat descriptors, one dma_start per 2-4 k-tiles), which keeps the single
DMA queue ahead of the PE from the first block on. Output stores issue
from the Activation-engine DGE queue so they never stall input loads.
"""

import os
import sys
import types

sys.path.insert(0, "/opt/trn_rl_repo")

import numpy as np

# trace=True under axon needs antenv.axon_hooks, absent from this image.
# Register the same ctypes-backed NTFF hook trn_boot would have installed.
if "antenv.axon_hooks" not in sys.modules:
    _m = types.ModuleType("antenv.axon_hooks")
    _m._hook = None

    def _set_hook(h):
        _m._hook = h

    def _get_hook():
        return _m._hook

    _m.set_axon_ntff_profile_hook = _set_hook
    _m.get_axon_ntff_profile_hook = _get_hook
    sys.modules["antenv.axon_hooks"] = _m
    try:
        from trn_agent_boot.trn_boot import _ntff_profile_via_ctypes

        _m.set_axon_ntff_profile_hook(
            _ntff_profile_via_ctypes("/opt/axon/libaxon_pjrt.so")
        )
    except Exception:
        pass

import concourse.bacc as bacc
import concourse.tile as tile
from concourse import mybir
from concourse.bass_utils import run_bass_kernel_spmd

N_CORES = 8
B, D, H = 8192, 1024, 1024
BL = B // N_CORES  # batch rows per core
P = 128  # partitions
NF = 512  # matmul free dim (one PSUM bank of fp32)
KD = D // P  # k-tiles per 1024 contraction
MT = BL // P  # batch m-tiles per core
HB = BL // 2  # batch half (stationary act tiles per half-group)
F32 = mybir.dt.float32
BF16 = mybir.dt.bfloat16

_CACHE = {}


def _build_program(with_bias):
    nc = bacc.Bacc("TRN2", target_bir_lowering=False, debug=False)

    # Host-retiled layouts (see kernel()): row index is (group*128 + p),
    # column index is (ktile*512 + c) so one partition's data for a span of
    # k-tiles is a single contiguous run.
    xt = nc.declare_dram_parameter("xt", [2 * P, KD * NF], BF16, isOutput=False)
    stt = nc.declare_dram_parameter("stt", [2 * P, KD * NF], BF16, isOutput=False)
    wzrt = nc.declare_dram_parameter("wzrt", [4 * P, 2 * KD * NF], BF16, isOutput=False)
    wxt = nc.declare_dram_parameter("wxt", [2 * P, KD * NF], BF16, isOutput=False)
    wht = nc.declare_dram_parameter("wht", [2 * P, KD * NF], BF16, isOutput=False)
    st = nc.declare_dram_parameter("st", [BL, H], F32, isOutput=False)
    if with_bias:
        bzr = nc.declare_dram_parameter("bzr", [1, 2 * H], BF16, isOutput=False)
        bx = nc.declare_dram_parameter("bx", [1, H], BF16, isOutput=False)
    out = nc.declare_dram_parameter("out", [BL, H], F32, isOutput=True)

    with tile.TileContext(nc) as tc:
        with (
            tc.tile_pool(name="acts", bufs=1) as acts,
            tc.tile_pool(name="wts", bufs=1) as wts,
            tc.tile_pool(name="stash", bufs=1) as stash,
            tc.tile_pool(name="stp", bufs=3) as stp,
            tc.tile_pool(name="tmp", bufs=3) as tmp,
            tc.tile_pool(name="small", bufs=1) as small,
            tc.tile_pool(name="ps", bufs=8, space="PSUM") as ps,
        ):
            # Warmup matmuls on scratch data ride out the PE p-state ramp and
            # HAM throttle while the first input DMAs land.
            warm_sb = small.tile([P, 2 * P], BF16, tag="warm_sb")
            nc.vector.memset(warm_sb, 0.0)
            warm_ps = ps.tile([P, 2 * P], F32, tag="ps", name="warm_ps")
            for i in range(14):
                nc.tensor.matmul(
                    warm_ps, warm_sb[:, :P], warm_sb, start=True, stop=True
                )

            if with_bias:
                ones = small.tile([1, P], BF16, tag="ones")
                nc.vector.memset(ones, 1.0)
                bzr_sb = small.tile([1, 2 * H], BF16, tag="bzr")
                nc.sync.dma_start(out=bzr_sb, in_=bzr.ap())
                bx_sb = small.tile([1, H], BF16, tag="bx")
                nc.sync.dma_start(out=bx_sb, in_=bx.ap())

            # Resident tiles: activations (stationary) and all weights (moving).
            xact = [acts.tile([P, KD * NF], BF16, tag=f"xa{h}", name=f"xa{h}") for h in range(2)]
            sact = [acts.tile([P, KD * NF], BF16, tag=f"sa{h}", name=f"sa{h}") for h in range(2)]
            wzr_sb = [wts.tile([P, 2 * KD * NF], BF16, tag=f"wzr{g}", name=f"wzr{g}") for g in range(4)]
            wx_sb = [wts.tile([P, KD * NF], BF16, tag=f"wx{g}", name=f"wx{g}") for g in range(2)]
            wh_sb = [wts.tile([P, KD * NF], BF16, tag=f"wh{g}", name=f"wh{g}") for g in range(2)]

            def dma_cols(dst_tile, src, grp, c0, c1):
                nc.sync.dma_start(
                    out=dst_tile[:, c0:c1],
                    in_=src.ap()[grp * P : (grp + 1) * P, c0:c1],
                )

            # DMA issue order = consumption order of zr_block(0) (k-tile j
            # needs wzr[g0] slice j plus x-act (j<8) or s-act (j-8) of the
            # half-group being processed), then the rest of the inputs.
            KC = 2 * NF  # two k-tiles per steady chunk
            for k in range(4):  # single k-tile chunks: fastest first matmul
                dma_cols(wzr_sb[0], wzrt, 0, k * NF, (k + 1) * NF)
                dma_cols(xact[0], xt, 0, k * NF, (k + 1) * NF)
            dma_cols(sact[0], stt, 0, 0 * KC, 1 * KC)     # s(h0) k0-1
            dma_cols(xact[0], xt, 0, 2 * KC, 3 * KC)      # x(h0) k4-5
            dma_cols(wzr_sb[0], wzrt, 0, 2 * KC, 3 * KC)  # w k4-5
            dma_cols(sact[0], stt, 0, 1 * KC, 2 * KC)     # s(h0) k2-3
            dma_cols(xact[0], xt, 0, 3 * KC, 4 * KC)      # x(h0) k6-7
            dma_cols(wzr_sb[0], wzrt, 0, 3 * KC, 4 * KC)  # w k6-7
            dma_cols(sact[0], stt, 0, 2 * KC, 3 * KC)     # s(h0) k4-5
            dma_cols(sact[0], stt, 0, 3 * KC, 4 * KC)     # s(h0) k6-7
            for kc in range(4, 8):                        # w k8-15
                dma_cols(wzr_sb[0], wzrt, 0, kc * KC, (kc + 1) * KC)
            # half-1 activations, then the remaining weights in use order.
            for h in range(2):
                dma_cols(xact[1], xt, 1, h * 4 * NF, (h + 1) * 4 * NF)
                dma_cols(sact[1], stt, 1, h * 4 * NF, (h + 1) * 4 * NF)
            for kc in range(4):                           # wzr g2 (r, c=0)
                dma_cols(wzr_sb[2], wzrt, 2, kc * 4 * NF, (kc + 1) * 4 * NF)
            for h in range(2):
                dma_cols(wx_sb[0], wxt, 0, h * 4 * NF, (h + 1) * 4 * NF)
                dma_cols(wh_sb[0], wht, 0, h * 4 * NF, (h + 1) * 4 * NF)
            for kc in range(4):                           # wzr g1 (z, c=1)
                dma_cols(wzr_sb[1], wzrt, 1, kc * 4 * NF, (kc + 1) * 4 * NF)
            for kc in range(4):                           # wzr g3 (r, c=1)
                dma_cols(wzr_sb[3], wzrt, 3, kc * 4 * NF, (kc + 1) * 4 * NF)
            for h in range(2):
                dma_cols(wx_sb[1], wxt, 1, h * 4 * NF, (h + 1) * 4 * NF)
                dma_cols(wh_sb[1], wht, 1, h * 4 * NF, (h + 1) * 4 * NF)

            def act_slice(k, m):
                t = xact if k < KD else sact
                kk = k % KD
                h, r = divmod(m, 4)
                return t[h][:, kk * NF + r * P : kk * NF + (r + 1) * P]

            # Half-column sigmoid stashes, reused across the two c-rounds.
            z_st = [stash.tile([P, NF], BF16, tag=f"z{m}", name=f"z{m}") for m in range(MT)]
            r_st = [stash.tile([P, NF], BF16, tag=f"r{m}", name=f"r{m}") for m in range(MT)]

            def zr_block(g, dst):
                """One 512-col block of the fused z/r GEMM: K=2048, k-outer /
                m-inner over half-groups of 4 PSUM banks; sigmoid into dst."""
                for half in range(2):
                    accs = []
                    for mi in range(4):
                        acc = ps.tile([P, NF], F32, tag="ps", name="acc")
                        accs.append(acc)
                        if with_bias:
                            nc.tensor.matmul(
                                acc,
                                ones,
                                bzr_sb[:, g * NF : (g + 1) * NF],
                                start=True,
                                stop=False,
                            )
                    for k in range(2 * KD):
                        wsl = wzr_sb[g][:, k * NF : (k + 1) * NF]
                        for mi in range(4):
                            m = half * 4 + mi
                            nc.tensor.matmul(
                                accs[mi],
                                act_slice(k, m),
                                wsl,
                                start=(k == 0 and not with_bias),
                                stop=(k == 2 * KD - 1),
                            )
                    for mi in range(4):
                        m = half * 4 + mi
                        nc.scalar.activation(
                            dst[m], accs[mi], mybir.ActivationFunctionType.Sigmoid
                        )

            for c in range(2):  # 512-wide column block of H
                csl = slice(c * NF, (c + 1) * NF)
                zr_block(c, z_st)       # z columns c*512..
                zr_block(2 + c, r_st)   # r columns c*512..

                # xh & hh GEMMs + fused gate epilogue for this column block
                for m in range(MT):
                    msl = slice(m * P, (m + 1) * P)
                    st_t = stp.tile([P, NF], F32, tag="st", name="st_t")
                    nc.sync.dma_start(out=st_t, in_=st.ap()[msl, csl])
                    # Precompute z*state and 1-z off the critical path: the
                    # post-tanh chain is then just mul+add.
                    zs = stp.tile([P, NF], BF16, tag="zs", name="zs")
                    nc.vector.tensor_mul(zs, z_st[m], st_t)
                    oz = stp.tile([P, NF], BF16, tag="oz", name="oz")
                    nc.scalar.activation(
                        oz, z_st[m], mybir.ActivationFunctionType.Copy,
                        bias=1.0, scale=-1.0,
                    )

                    phh = ps.tile([P, NF], F32, tag="ps", name="phh")
                    for k in range(KD):
                        nc.tensor.matmul(
                            phh,
                            act_slice(KD + k, m),
                            wh_sb[c][:, k * NF : (k + 1) * NF],
                            start=(k == 0),
                            stop=(k == KD - 1),
                        )
                    pxh = ps.tile([P, NF], F32, tag="ps", name="pxh")
                    if with_bias:
                        nc.tensor.matmul(
                            pxh, ones, bx_sb[:, csl], start=True, stop=False
                        )
                    for k in range(KD):
                        nc.tensor.matmul(
                            pxh,
                            act_slice(k, m),
                            wx_sb[c][:, k * NF : (k + 1) * NF],
                            start=(k == 0 and not with_bias),
                            stop=(k == KD - 1),
                        )

                    # h_ = tanh(xh + hh*r); hid = h_ + z*(state - h_)
                    # Later chunks shrink so the post-matmul drain chain at
                    # kernel end stays short.
                    t = tmp.tile([P, NF], F32, tag="t", name="t")
                    h = tmp.tile([P, NF], BF16, tag="h", name="h")
                    h2 = tmp.tile([P, NF], BF16, tag="h2", name="h2")
                    nchunk = 2 if (c == 1 and m >= MT - 2) else 1
                    cw = NF // nchunk
                    for q in range(nchunk):
                        qs = slice(q * cw, (q + 1) * cw)
                        nc.vector.tensor_mul(t[:, qs], phh[:, qs], r_st[m][:, qs])
                        nc.vector.tensor_add(t[:, qs], t[:, qs], pxh[:, qs])
                        nc.scalar.activation(
                            h[:, qs], t[:, qs], mybir.ActivationFunctionType.Tanh
                        )
                        nc.vector.tensor_mul(h2[:, qs], h[:, qs], oz[:, qs])
                        nc.vector.tensor_add(t[:, qs], h2[:, qs], zs[:, qs])
                        nc.sync.dma_start(
                            out=out.ap()[msl, c * NF + q * cw : c * NF + (q + 1) * cw],
                            in_=t[:, qs],
                        )

    nc.compile()
    return nc


def _get_program(with_bias):
    key = ("nc", with_bias)
    if key not in _CACHE:
        _CACHE[key] = _build_program(with_bias)
    return _CACHE[key]


def _retile(w, ngrp):
    """[K, N] -> [ngrp*128 + p, ktile*512 + c] with w[k*128+p, g*512+c] at
    [g*128+p, k*512+c]; one partition's k-span is contiguous."""
    kt = w.shape[0] // P
    return np.ascontiguousarray(
        w.reshape(kt, P, ngrp, NF).transpose(2, 1, 0, 3).reshape(ngrp * P, kt * NF)
    )


def kernel(inp, state, wx, bx, wh, wr, ur, uz, wz, br, bz):
    import ml_dtypes

    bf16 = ml_dtypes.bfloat16
    inp = np.asarray(inp, dtype=np.float32)
    state = np.asarray(state, dtype=np.float32)
    w_zr = np.block(
        [
            [np.asarray(wz, np.float32), np.asarray(wr, np.float32)],
            [np.asarray(uz, np.float32), np.asarray(ur, np.float32)],
        ]
    ).astype(bf16)
    w_x = np.asarray(wx, np.float32).astype(bf16)
    w_h = np.asarray(wh, np.float32).astype(bf16)
    b_zr = np.concatenate(
        [np.asarray(bz, np.float32), np.asarray(br, np.float32)]
    )[None, :].astype(bf16)
    b_x = np.ascontiguousarray(np.asarray(bx, np.float32).astype(bf16))[None, :]

    wzr_t = _retile(w_zr, 4)
    wx_t = _retile(w_x, 2)
    wh_t = _retile(w_h, 2)

    with_bias = bool(np.any(b_zr) or np.any(b_x))
    in_maps = []
    for cidx in range(N_CORES):
        sl = slice(cidx * BL, (cidx + 1) * BL)
        xT = inp[sl].T.astype(bf16)  # [D, BL]
        sT = state[sl].T.astype(bf16)  # [H, BL]
        im = {
            "xt": _retile(xT, 2),
            "stt": _retile(sT, 2),
            "st": np.ascontiguousarray(state[sl]),
            "wzrt": wzr_t,
            "wxt": wx_t,
            "wht": wh_t,
        }
        if with_bias:
            im["bzr"] = b_zr
            im["bx"] = b_x
        in_maps.append(im)

    nc = _get_program(with_bias)
    trace = bool(int(os.environ.get("GRU_TRACE", "0")))
    res = run_bass_kernel_spmd(nc, in_maps, list(range(N_CORES)), trace=trace)
    if trace:
        _CACHE["last_exec_time_ns"] = res.exec_time_ns
        _CACHE["last_results"] = res
    return np.concatenate([res.results[c]["out"] for c in range(N_CORES)], axis=0)


# revision 10
# speedup vs baseline: 1.0224x; 1.0224x over previous
"""GRU cell kernel for Trainium2, data-parallel over batch across 8 NeuronCores.

Reference computation (B=8192, D=H=1024), per batch row:
    z = sigmoid(inp@wz + state@uz + bz)
    r = sigmoid(inp@wr + state@ur + br)
    h_ = tanh(inp@wx + bx + (state@wh) * r)
    hid = (1-z)*h_ + state*z

Strategy: each core takes a 1024-row batch shard. The z/r projections fuse
into one [1024,2048]@[2048,2048] GEMM (act = [inp|state], W = [[wz,wr],[uz,ur]]).
xh and hh stay separate GEMMs because hh is gated by r before the sum.
All matmul operands are bf16 (fp32 PSUM accumulate): the moving operand
streams one 512-wide column per PE cycle and the bf16 LDWEIGHTS (~97ns)
hides under the previous matmul, so the PE runs at the 213ns/matmul
streaming floor. Weights and transposed activations are retiled on the
host so every DMA lands as a few large contiguous runs per partition
(fat descriptors, one dma_start per 2-4 k-tiles), which keeps the single
DMA queue ahead of the PE from the first block on. Output stores issue
from the Activation-engine DGE queue so they never stall input loads.
"""

import os
import sys
import types

sys.path.insert(0, "/opt/trn_rl_repo")

import numpy as np

# trace=True under axon needs antenv.axon_hooks, absent from this image.
# Register the same ctypes-backed NTFF hook trn_boot would have installed.
if "antenv.axon_hooks" not in sys.modules:
    _m = types.ModuleType("antenv.axon_hooks")
    _m._hook = None

    def _set_hook(h):
        _m._hook = h

    def _get_hook():
        return _m._hook

    _m.set_axon_ntff_profile_hook = _set_hook
    _m.get_axon_ntff_profile_hook = _get_hook
    sys.modules["antenv.axon_hooks"] = _m
    try:
        from trn_agent_boot.trn_boot import _ntff_profile_via_ctypes

        _m.set_axon_ntff_profile_hook(
            _ntff_profile_via_ctypes("/opt/axon/libaxon_pjrt.so")
        )
    except Exception:
        pass

import concourse.bacc as bacc
import concourse.tile as tile
from concourse import mybir
from concourse.bass_utils import run_bass_kernel_spmd

N_CORES = 8
B, D, H = 8192, 1024, 1024
BL = B // N_CORES  # batch rows per core
P = 128  # partitions
NF = 512  # matmul free dim (one PSUM bank of fp32)
KD = D // P  # k-tiles per 1024 contraction
MT = BL // P  # batch m-tiles per core
HB = BL // 2  # batch half (stationary act tiles per half-group)
F32 = mybir.dt.float32
BF16 = mybir.dt.bfloat16

_CACHE = {}


def _build_program(with_bias):
    nc = bacc.Bacc("TRN2", target_bir_lowering=False, debug=False)

    # Host-retiled layouts (see kernel()): row index is (group*128 + p),
    # column index is (ktile*512 + c) so one partition's data for a span of
    # k-tiles is a single contiguous run.
    xt = nc.declare_dram_parameter("xt", [2 * P, KD * NF], BF16, isOutput=False)
    stt = nc.declare_dram_parameter("stt", [2 * P, KD * NF], BF16, isOutput=False)
    wzrt = nc.declare_dram_parameter("wzrt", [4 * P, 2 * KD * NF], BF16, isOutput=False)
    wxt = nc.declare_dram_parameter("wxt", [2 * P, KD * NF], BF16, isOutput=False)
    wht = nc.declare_dram_parameter("wht", [2 * P, KD * NF], BF16, isOutput=False)
    st = nc.declare_dram_parameter("st", [BL, H], F32, isOutput=False)
    if with_bias:
        bzr = nc.declare_dram_parameter("bzr", [1, 2 * H], BF16, isOutput=False)
        bx = nc.declare_dram_parameter("bx", [1, H], BF16, isOutput=False)
    out = nc.declare_dram_parameter("out", [BL, H], F32, isOutput=True)

    with tile.TileContext(nc) as tc:
        with (
            tc.tile_pool(name="acts", bufs=1) as acts,
            tc.tile_pool(name="wts", bufs=1) as wts,
            tc.tile_pool(name="stash", bufs=1) as stash,
            tc.tile_pool(name="stp", bufs=3) as stp,
            tc.tile_pool(name="tmp", bufs=3) as tmp,
            tc.tile_pool(name="small", bufs=1) as small,
            tc.tile_pool(name="ps", bufs=8, space="PSUM") as ps,
        ):
            # Warmup matmuls on scratch data ride out the PE p-state ramp and
            # HAM throttle while the first input DMAs land.
            warm_sb = small.tile([P, 2 * P], BF16, tag="warm_sb")
            nc.vector.memset(warm_sb, 0.0)
            warm_ps = ps.tile([P, 2 * P], F32, tag="ps", name="warm_ps")
            for i in range(11):
                nc.tensor.matmul(
                    warm_ps, warm_sb[:, :P], warm_sb, start=True, stop=True
                )

            if with_bias:
                ones = small.tile([1, P], BF16, tag="ones")
                nc.vector.memset(ones, 1.0)
                bzr_sb = small.tile([1, 2 * H], BF16, tag="bzr")
                nc.sync.dma_start(out=bzr_sb, in_=bzr.ap())
                bx_sb = small.tile([1, H], BF16, tag="bx")
                nc.sync.dma_start(out=bx_sb, in_=bx.ap())

            # Resident tiles: activations (stationary) and all weights (moving).
            xact = [acts.tile([P, KD * NF], BF16, tag=f"xa{h}", name=f"xa{h}") for h in range(2)]
            sact = [acts.tile([P, KD * NF], BF16, tag=f"sa{h}", name=f"sa{h}") for h in range(2)]
            wzr_sb = [wts.tile([P, 2 * KD * NF], BF16, tag=f"wzr{g}", name=f"wzr{g}") for g in range(4)]
            wx_sb = [wts.tile([P, KD * NF], BF16, tag=f"wx{g}", name=f"wx{g}") for g in range(2)]
            wh_sb = [wts.tile([P, KD * NF], BF16, tag=f"wh{g}", name=f"wh{g}") for g in range(2)]

            def dma_cols(dst_tile, src, grp, c0, c1):
                nc.sync.dma_start(
                    out=dst_tile[:, c0:c1],
                    in_=src.ap()[grp * P : (grp + 1) * P, c0:c1],
                )

            # DMA issue order = consumption order of zr_block(0): k-tile j
            # needs wzr[g0] slice j plus x-act (j<8) or s-act (j-8). Pairing
            # (w_j, act_j) keeps supply exactly one k-tile ahead of the PE.
            # Later tiles ship as whole-group DMAs: the host retiling makes
            # each partition's span one contiguous 8-16KB descriptor, which
            # the DMA engines move at full HBM rate.
            for j in (0, 1):  # single k-tile chunks: fastest first matmul
                dma_cols(wzr_sb[0], wzrt, 0, j * NF, (j + 1) * NF)
                dma_cols(xact[0], xt, 0, j * NF, (j + 1) * NF)
            for jc in range(1, 4):  # k2-3, k4-5, k6-7 pairs
                dma_cols(wzr_sb[0], wzrt, 0, jc * 2 * NF, (jc + 1) * 2 * NF)
                dma_cols(xact[0], xt, 0, jc * 2 * NF, (jc + 1) * 2 * NF)
            for jc in range(4):  # k8-15: w pairs with s-acts k0-7
                dma_cols(wzr_sb[0], wzrt, 0, (4 + jc) * 2 * NF, (5 + jc) * 2 * NF)
                dma_cols(sact[0], stt, 0, jc * 2 * NF, (jc + 1) * 2 * NF)
            # half-1 activations (4-ktile chunks), then whole-group weights.
            for h in range(2):
                dma_cols(xact[1], xt, 1, h * 4 * NF, (h + 1) * 4 * NF)
            for h in range(2):
                dma_cols(sact[1], stt, 1, h * 4 * NF, (h + 1) * 4 * NF)
            dma_cols(wzr_sb[2], wzrt, 2, 0, 2 * KD * NF)   # r, c=0
            dma_cols(wx_sb[0], wxt, 0, 0, KD * NF)
            dma_cols(wh_sb[0], wht, 0, 0, KD * NF)
            dma_cols(wzr_sb[1], wzrt, 1, 0, 2 * KD * NF)   # z, c=1
            dma_cols(wzr_sb[3], wzrt, 3, 0, 2 * KD * NF)   # r, c=1
            dma_cols(wx_sb[1], wxt, 1, 0, KD * NF)
            dma_cols(wh_sb[1], wht, 1, 0, KD * NF)

            def act_slice(k, m):
                t = xact if k < KD else sact
                kk = k % KD
                h, r = divmod(m, 4)
                return t[h][:, kk * NF + r * P : kk * NF + (r + 1) * P]

            # Half-column sigmoid stashes, reused across the two c-rounds.
            z_st = [stash.tile([P, NF], BF16, tag=f"z{m}", name=f"z{m}") for m in range(MT)]
            r_st = [stash.tile([P, NF], BF16, tag=f"r{m}", name=f"r{m}") for m in range(MT)]

            def zr_block(g, dst):
                """One 512-col block of the fused z/r GEMM: K=2048, k-outer /
                m-inner over half-groups of 4 PSUM banks; sigmoid into dst."""
                for half in range(2):
                    accs = []
                    for mi in range(4):
                        acc = ps.tile([P, NF], F32, tag="ps", name="acc")
                        accs.append(acc)
                        if with_bias:
                            nc.tensor.matmul(
                                acc,
                                ones,
                                bzr_sb[:, g * NF : (g + 1) * NF],
                                start=True,
                                stop=False,
                            )
                    for k in range(2 * KD):
                        wsl = wzr_sb[g][:, k * NF : (k + 1) * NF]
                        for mi in range(4):
                            m = half * 4 + mi
                            nc.tensor.matmul(
                                accs[mi],
                                act_slice(k, m),
                                wsl,
                                start=(k == 0 and not with_bias),
                                stop=(k == 2 * KD - 1),
                            )
                    for mi in range(4):
                        m = half * 4 + mi
                        nc.scalar.activation(
                            dst[m], accs[mi], mybir.ActivationFunctionType.Sigmoid
                        )

            for c in range(2):  # 512-wide column block of H
                csl = slice(c * NF, (c + 1) * NF)
                zr_block(c, z_st)       # z columns c*512..
                zr_block(2 + c, r_st)   # r columns c*512..

                # xh & hh GEMMs + fused gate epilogue for this column block
                for m in range(MT):
                    msl = slice(m * P, (m + 1) * P)
                    st_t = stp.tile([P, NF], F32, tag="st", name="st_t")
                    nc.sync.dma_start(out=st_t, in_=st.ap()[msl, csl])
                    # Precompute z*state and 1-z off the critical path: the
                    # post-tanh chain is then just mul+add.
                    zs = stp.tile([P, NF], BF16, tag="zs", name="zs")
                    nc.vector.tensor_mul(zs, z_st[m], st_t)
                    oz = stp.tile([P, NF], BF16, tag="oz", name="oz")
                    nc.scalar.activation(
                        oz, z_st[m], mybir.ActivationFunctionType.Copy,
                        bias=1.0, scale=-1.0,
                    )

                    phh = ps.tile([P, NF], F32, tag="ps", name="phh")
                    for k in range(KD):
                        nc.tensor.matmul(
                            phh,
                            act_slice(KD + k, m),
                            wh_sb[c][:, k * NF : (k + 1) * NF],
                            start=(k == 0),
                            stop=(k == KD - 1),
                        )
                    pxh = ps.tile([P, NF], F32, tag="ps", name="pxh")
                    if with_bias:
                        nc.tensor.matmul(
                            pxh, ones, bx_sb[:, csl], start=True, stop=False
                        )
                    for k in range(KD):
                        nc.tensor.matmul(
                            pxh,
                            act_slice(k, m),
                            wx_sb[c][:, k * NF : (k + 1) * NF],
                            start=(k == 0 and not with_bias),
                            stop=(k == KD - 1),
                        )

                    # h_ = tanh(xh + hh*r); hid = h_ + z*(state - h_)
                    # Later chunks shrink so the post-matmul drain chain at
                    # kernel end stays short.
                    t = tmp.tile([P, NF], F32, tag="t", name="t")
                    h = tmp.tile([P, NF], BF16, tag="h", name="h")
                    h2 = tmp.tile([P, NF], BF16, tag="h2", name="h2")
                    nchunk = 2 if (c == 1 and m >= MT - 2) else 1
                    cw = NF // nchunk
                    for q in range(nchunk):
                        qs = slice(q * cw, (q + 1) * cw)
                        nc.vector.tensor_mul(t[:, qs], phh[:, qs], r_st[m][:, qs])
                        nc.vector.tensor_add(t[:, qs], t[:, qs], pxh[:, qs])
                        nc.scalar.activation(
                            h[:, qs], t[:, qs], mybir.ActivationFunctionType.Tanh
                        )
                        nc.vector.tensor_mul(h2[:, qs], h[:, qs], oz[:, qs])
                        nc.vector.tensor_add(t[:, qs], h2[:, qs], zs[:, qs])
                        nc.sync.dma_start(
                            out=out.ap()[msl, c * NF + q * cw : c * NF + (q + 1) * cw],
                            in_=t[:, qs],
                        )

    nc.compile()
    return nc


def _get_program(with_bias):
    key = ("nc", with_bias)
    if key not in _CACHE:
        _CACHE[key] = _build_program(with_bias)
    return _CACHE[key]


def _retile(w, ngrp):
    """[K, N] -> [ngrp*128 + p, ktile*512 + c] with w[k*128+p, g*512+c] at
    [g*128+p, k*512+c]; one partition's k-span is contiguous."""
    kt = w.shape[0] // P
    return np.ascontiguousarray(
        w.reshape(kt, P, ngrp, NF).transpose(2, 1, 0, 3).reshape(ngrp * P, kt * NF)
    )


def kernel(inp, state, wx, bx, wh, wr, ur, uz, wz, br, bz):
    import ml_dtypes

    bf16 = ml_dtypes.bfloat16
    inp = np.asarray(inp, dtype=np.float32)
    state = np.asarray(state, dtype=np.float32)
    w_zr = np.block(
        [
            [np.asarray(wz, np.float32), np.asarray(wr, np.float32)],
            [np.asarray(uz, np.float32), np.asarray(ur, np.float32)],
        ]
    ).astype(bf16)
    w_x = np.asarray(wx, np.float32).astype(bf16)
    w_h = np.asarray(wh, np.float32).astype(bf16)
    b_zr = np.concatenate(
        [np.asarray(bz, np.float32), np.asarray(br, np.float32)]
    )[None, :].astype(bf16)
    b_x = np.ascontiguousarray(np.asarray(bx, np.float32).astype(bf16))[None, :]

    wzr_t = _retile(w_zr, 4)
    wx_t = _retile(w_x, 2)
    wh_t = _retile(w_h, 2)

    with_bias = bool(np.any(b_zr) or np.any(b_x))
    in_maps = []
    for cidx in range(N_CORES):
        sl = slice(cidx * BL, (cidx + 1) * BL)
        xT = inp[sl].T.astype(bf16)  # [D, BL]
        sT = state[sl].T.astype(bf16)  # [H, BL]
        im = {
            "xt": _retile(xT, 2),
            "stt": _retile(sT, 2),
            "st": np.ascontiguousarray(state[sl]),
            "wzrt": wzr_t,
            "wxt": wx_t,
            "wht": wh_t,
        }
        if with_bias:
            im["bzr"] = b_zr
            im["bx"] = b_x
        in_maps.append(im)

    nc = _get_program(with_bias)
    trace = bool(int(os.environ.get("GRU_TRACE", "0")))
    res = run_bass_kernel_spmd(nc, in_maps, list(range(N_CORES)), trace=trace)
    if trace:
        _CACHE["last_exec_time_ns"] = res.exec_time_ns
        _CACHE["last_results"] = res
    return np.concatenate([res.results[c]["out"] for c in range(N_CORES)], axis=0)


# revision 11
# speedup vs baseline: 1.4401x; 1.4086x over previous
"""GRU cell kernel for Trainium2, data-parallel over batch across 8 NeuronCores.

Reference computation (B=8192, D=H=1024), per batch row:
    z = sigmoid(inp@wz + state@uz + bz)
    r = sigmoid(inp@wr + state@ur + br)
    h_ = tanh(inp@wx + bx + (state@wh) * r)
    hid = (1-z)*h_ + state*z

Strategy: each core takes a 1024-row batch shard. The z/r projections fuse
into one [1024,2048]@[2048,2048] GEMM computed in fp8(e4m3) DoubleRow mode:
one PE instruction contracts TWO 128-deep k-tiles in the time a bf16
instruction contracts one, halving the zr GEMM's instruction count. The
sigmoid nonlinearity absorbs the fp8 quantization noise (measured 1.4e-2
output rel err vs the 2e-2 budget). xh and hh stay bf16 GEMMs (the tanh
path needs the precision). All PSUM accumulation is fp32; the fp8 scales
(x*32, w*1600) divide out inside the sigmoid's activation-scale. Weights
and activations are retiled on the host so each DMA lands as few large
contiguous runs per partition, issued in consumption order. The post-tanh
chain is shortened by precomputing z*state and (1-z) off the critical path.
"""

import os
import sys
import types

sys.path.insert(0, "/opt/trn_rl_repo")

import numpy as np

# trace=True under axon needs antenv.axon_hooks, absent from this image.
# Register the same ctypes-backed NTFF hook trn_boot would have installed.
if "antenv.axon_hooks" not in sys.modules:
    _m = types.ModuleType("antenv.axon_hooks")
    _m._hook = None

    def _set_hook(h):
        _m._hook = h

    def _get_hook():
        return _m._hook

    _m.set_axon_ntff_profile_hook = _set_hook
    _m.get_axon_ntff_profile_hook = _get_hook
    sys.modules["antenv.axon_hooks"] = _m
    try:
        from trn_agent_boot.trn_boot import _ntff_profile_via_ctypes

        _m.set_axon_ntff_profile_hook(
            _ntff_profile_via_ctypes("/opt/axon/libaxon_pjrt.so")
        )
    except Exception:
        pass

import concourse.bacc as bacc
import concourse.tile as tile
from concourse import mybir
from concourse.bass_utils import run_bass_kernel_spmd

N_CORES = 8
B, D, H = 8192, 1024, 1024
BL = B // N_CORES  # batch rows per core
P = 128  # partitions
NF = 512  # matmul free dim (one PSUM bank of fp32)
KD = D // P  # k-tiles per 1024 contraction
MT = BL // P  # batch m-tiles per core
F32 = mybir.dt.float32
BF16 = mybir.dt.bfloat16
FP8 = mybir.dt.float8e4
DR = mybir.MatmulPerfMode.DoubleRow
SX, SW = 32.0, 1600.0  # fp8 quantization scales for acts / weights

_CACHE = {}


def _build_program(with_bias):
    nc = bacc.Bacc("TRN2", target_bir_lowering=False, debug=False)

    # Host-retiled layouts (see kernel()): row index is (group*128 + p),
    # then [ktile, col] so one partition's k-span is one contiguous run.
    xt = nc.declare_dram_parameter("xt", [2 * P, KD * NF], BF16, isOutput=False)
    stt = nc.declare_dram_parameter("stt", [2 * P, KD * NF], BF16, isOutput=False)
    wxt = nc.declare_dram_parameter("wxt", [2 * P, KD * NF], BF16, isOutput=False)
    wht = nc.declare_dram_parameter("wht", [2 * P, KD * NF], BF16, isOutput=False)
    st = nc.declare_dram_parameter("st", [BL, H], F32, isOutput=False)
    if with_bias:
        # Bias fallback: zr GEMM in bf16 with rank-1 bias rows (the fp8
        # PSUM pre-scale can't host an unscaled bias cleanly).
        wzrt = nc.declare_dram_parameter(
            "wzrt", [4 * P, 2 * KD * NF], BF16, isOutput=False
        )
        bzr = nc.declare_dram_parameter("bzr", [1, 2 * H], BF16, isOutput=False)
        bx = nc.declare_dram_parameter("bx", [1, H], BF16, isOutput=False)
    else:
        x8d = nc.declare_dram_parameter("x8", [2 * P, KD, NF], FP8, isOutput=False)
        s8d = nc.declare_dram_parameter("s8", [2 * P, KD, NF], FP8, isOutput=False)
        wzr8d = nc.declare_dram_parameter(
            "wzr8", [4 * P, 2 * KD, NF], FP8, isOutput=False
        )
    out = nc.declare_dram_parameter("out", [BL, H], F32, isOutput=True)

    with tile.TileContext(nc) as tc:
        with (
            tc.tile_pool(name="acts", bufs=1) as acts,
            tc.tile_pool(name="wts", bufs=1) as wts,
            tc.tile_pool(name="stash", bufs=1) as stash,
            tc.tile_pool(name="stp", bufs=8) as stp,
            tc.tile_pool(name="tmp", bufs=3) as tmp,
            tc.tile_pool(name="small", bufs=1) as small,
            tc.tile_pool(name="ps", bufs=8, space="PSUM") as ps,
        ):
            # Warmup matmuls on scratch data ride out the PE p-state ramp and
            # HAM throttle while the first input DMAs land.
            warm_sb = small.tile([P, 2 * P], BF16, tag="warm_sb")
            nc.vector.memset(warm_sb, 0.0)
            warm_ps = ps.tile([P, 2 * P], F32, tag="ps", name="warm_ps")
            for i in range(11):
                nc.tensor.matmul(
                    warm_ps, warm_sb[:, :P], warm_sb, start=True, stop=True
                )

            if with_bias:
                ones = small.tile([1, P], BF16, tag="ones")
                nc.vector.memset(ones, 1.0)
                bzr_sb = small.tile([1, 2 * H], BF16, tag="bzr")
                nc.sync.dma_start(out=bzr_sb, in_=bzr.ap())
                bx_sb = small.tile([1, H], BF16, tag="bx")
                nc.sync.dma_start(out=bx_sb, in_=bx.ap())

            # Resident tiles. bf16 acts serve as xh/hh stationaries; fp8
            # copies (3D: [p, ktile, col]) serve the DoubleRow zr GEMM.
            xact = [acts.tile([P, KD * NF], BF16, tag=f"xa{h}", name=f"xa{h}") for h in range(2)]
            sact = [acts.tile([P, KD * NF], BF16, tag=f"sa{h}", name=f"sa{h}") for h in range(2)]
            wx_sb = [wts.tile([P, KD * NF], BF16, tag=f"wx{g}", name=f"wx{g}") for g in range(2)]
            wh_sb = [wts.tile([P, KD * NF], BF16, tag=f"wh{g}", name=f"wh{g}") for g in range(2)]
            if with_bias:
                wzr_sb = [
                    wts.tile([P, 2 * KD * NF], BF16, tag=f"wzr{g}", name=f"wzr{g}")
                    for g in range(4)
                ]
            else:
                x8a = [acts.tile([P, KD, NF], FP8, tag=f"x8{h}", name=f"x8{h}") for h in range(2)]
                s8a = [acts.tile([P, KD, NF], FP8, tag=f"s8{h}", name=f"s8{h}") for h in range(2)]
                wzr8_sb = [
                    wts.tile([P, 2 * KD, NF], FP8, tag=f"wz8{g}", name=f"wz8{g}")
                    for g in range(4)
                ]

            def dma_cols(dst_tile, src, grp, c0, c1):
                nc.sync.dma_start(
                    out=dst_tile[:, c0:c1],
                    in_=src.ap()[grp * P : (grp + 1) * P, c0:c1],
                )

            def dma3(dst_tile, src, grp, k0, k1):
                nc.sync.dma_start(
                    out=dst_tile[:, k0:k1, :],
                    in_=src.ap()[grp * P : (grp + 1) * P, k0:k1, :],
                )

            # DMA issue order = consumption order. zr_block(0) consumes k-tile
            # PAIR j as (wzr8 pair j, x8 pair j [j<4] or s8 pair j-4).
            if with_bias:
                for j in (0, 1):
                    dma_cols(wzr_sb[0], wzrt, 0, j * NF, (j + 1) * NF)
                    dma_cols(xact[0], xt, 0, j * NF, (j + 1) * NF)
                for jc in range(1, 4):
                    dma_cols(wzr_sb[0], wzrt, 0, jc * 2 * NF, (jc + 1) * 2 * NF)
                    dma_cols(xact[0], xt, 0, jc * 2 * NF, (jc + 1) * 2 * NF)
                for jc in range(4):
                    dma_cols(wzr_sb[0], wzrt, 0, (4 + jc) * 2 * NF, (5 + jc) * 2 * NF)
                    dma_cols(sact[0], stt, 0, jc * 2 * NF, (jc + 1) * 2 * NF)
                for h in range(2):
                    dma_cols(xact[1], xt, 1, h * 4 * NF, (h + 1) * 4 * NF)
                for h in range(2):
                    dma_cols(sact[1], stt, 1, h * 4 * NF, (h + 1) * 4 * NF)
                dma_cols(wzr_sb[2], wzrt, 2, 0, 2 * KD * NF)
                dma_cols(wx_sb[0], wxt, 0, 0, KD * NF)
                dma_cols(wh_sb[0], wht, 0, 0, KD * NF)
                dma_cols(wzr_sb[1], wzrt, 1, 0, 2 * KD * NF)
                dma_cols(wzr_sb[3], wzrt, 3, 0, 2 * KD * NF)
                dma_cols(wx_sb[1], wxt, 1, 0, KD * NF)
                dma_cols(wh_sb[1], wht, 1, 0, KD * NF)
            else:
                for j in range(4):  # single-pair chunks: w pair j + x pair j
                    dma3(wzr8_sb[0], wzr8d, 0, 2 * j, 2 * j + 2)
                    dma3(x8a[0], x8d, 0, 2 * j, 2 * j + 2)
                for j in range(2):  # w pairs 4-7 with s8(h0)
                    dma3(wzr8_sb[0], wzr8d, 0, 8 + 4 * j, 12 + 4 * j)
                    dma3(s8a[0], s8d, 0, 4 * j, 4 * j + 4)
                dma3(x8a[1], x8d, 1, 0, KD)      # h1 acts
                dma3(s8a[1], s8d, 1, 0, KD)
                dma3(wzr8_sb[2], wzr8d, 2, 0, 2 * KD)   # r, c=0
                # bf16 inputs for the c=0 xh/hh phase: hh (s-acts, wh) first.
                for h in range(2):
                    dma_cols(sact[h], stt, h, 0, KD * NF)
                dma_cols(wh_sb[0], wht, 0, 0, KD * NF)
                for h in range(2):
                    dma_cols(xact[h], xt, h, 0, KD * NF)
                dma_cols(wx_sb[0], wxt, 0, 0, KD * NF)

            def act_slice(k, m):
                t = xact if k < KD else sact
                kk = k % KD
                h, r = divmod(m, 4)
                return t[h][:, kk * NF + r * P : kk * NF + (r + 1) * P]

            # Half-column sigmoid stashes, reused across the two c-rounds.
            z_st = [stash.tile([P, NF], BF16, tag=f"z{m}", name=f"z{m}") for m in range(MT)]
            r_st = [stash.tile([P, NF], BF16, tag=f"r{m}", name=f"r{m}") for m in range(MT)]

            def zr_block_fp8(g, dst):
                """One 512-col block of the fused z/r GEMM in fp8 DoubleRow:
                8 k-pair instructions contract K=2048; sigmoid (with the
                fp8 descale folded into its input scale) into dst."""
                for half in range(2):
                    accs = []
                    for mi in range(4):
                        acc = ps.tile([P, NF], F32, tag="ps", name="acc")
                        accs.append(acc)
                    for j in range(KD):
                        rhs = wzr8_sb[g][:, 2 * j : 2 * j + 2, :]
                        a = (x8a if j < 4 else s8a)[half]
                        jj = j % 4
                        for mi in range(4):
                            lhsT = a[:, 2 * jj : 2 * jj + 2, mi * P : (mi + 1) * P]
                            nc.tensor.matmul(
                                accs[mi],
                                lhsT,
                                rhs,
                                start=(j == 0),
                                stop=(j == KD - 1),
                                perf_mode=DR,
                            )
                    for mi in range(4):
                        m = half * 4 + mi
                        nc.scalar.activation(
                            dst[m],
                            accs[mi],
                            mybir.ActivationFunctionType.Sigmoid,
                            scale=1.0 / (SX * SW),
                        )

            def zr_block_bf16(g, dst):
                for half in range(2):
                    accs = []
                    for mi in range(4):
                        acc = ps.tile([P, NF], F32, tag="ps", name="acc")
                        accs.append(acc)
                        nc.tensor.matmul(
                            acc,
                            ones,
                            bzr_sb[:, g * NF : (g + 1) * NF],
                            start=True,
                            stop=False,
                        )
                    for k in range(2 * KD):
                        wsl = wzr_sb[g][:, k * NF : (k + 1) * NF]
                        for mi in range(4):
                            m = half * 4 + mi
                            nc.tensor.matmul(
                                accs[mi], act_slice(k, m), wsl, start=False,
                                stop=(k == 2 * KD - 1),
                            )
                    for mi in range(4):
                        m = half * 4 + mi
                        nc.scalar.activation(
                            dst[m], accs[mi], mybir.ActivationFunctionType.Sigmoid
                        )

            zr_block = zr_block_bf16 if with_bias else zr_block_fp8

            for c in range(2):  # 512-wide column block of H
                csl = slice(c * NF, (c + 1) * NF)
                zr_block(c, z_st)       # z columns c*512..
                zr_block(2 + c, r_st)   # r columns c*512..
                if c == 0 and not with_bias:
                    # c=1 zr weights: issued here so they queue behind only
                    # the c=0 inputs and land before the c=1 round starts.
                    dma3(wzr8_sb[1], wzr8d, 1, 0, 2 * KD)
                    dma3(wzr8_sb[3], wzr8d, 3, 0, 2 * KD)

                # xh & hh GEMMs + fused gate epilogue for this column block
                for m in range(MT):
                    msl = slice(m * P, (m + 1) * P)
                    st_t = stp.tile([P, NF], F32, tag="st", name="st_t")
                    nc.sync.dma_start(out=st_t, in_=st.ap()[msl, csl])
                    # Precompute z*state and 1-z off the critical path: the
                    # post-tanh chain is then just mul+add.
                    zs = stp.tile([P, NF], BF16, tag="zs", name="zs")
                    nc.vector.tensor_mul(zs, z_st[m], st_t)
                    oz = stp.tile([P, NF], BF16, tag="oz", name="oz")
                    nc.scalar.activation(
                        oz, z_st[m], mybir.ActivationFunctionType.Copy,
                        bias=1.0, scale=-1.0,
                    )

                    phh = ps.tile([P, NF], F32, tag="ps", name="phh")
                    for k in range(KD):
                        nc.tensor.matmul(
                            phh,
                            act_slice(KD + k, m),
                            wh_sb[c][:, k * NF : (k + 1) * NF],
                            start=(k == 0),
                            stop=(k == KD - 1),
                        )
                    pxh = ps.tile([P, NF], F32, tag="ps", name="pxh")
                    if with_bias:
                        nc.tensor.matmul(
                            pxh, ones, bx_sb[:, csl], start=True, stop=False
                        )
                    for k in range(KD):
                        nc.tensor.matmul(
                            pxh,
                            act_slice(k, m),
                            wx_sb[c][:, k * NF : (k + 1) * NF],
                            start=(k == 0 and not with_bias),
                            stop=(k == KD - 1),
                        )

                    # h_ = tanh(xh + hh*r); hid = h_*(1-z) + z*state
                    t = tmp.tile([P, NF], F32, tag="t", name="t")
                    h = tmp.tile([P, NF], BF16, tag="h", name="h")
                    h2 = tmp.tile([P, NF], BF16, tag="h2", name="h2")
                    nchunk = 2 if (c == 1 and m >= MT - 2) else 1
                    cw = NF // nchunk
                    for q in range(nchunk):
                        qs = slice(q * cw, (q + 1) * cw)
                        nc.vector.tensor_mul(t[:, qs], phh[:, qs], r_st[m][:, qs])
                        nc.vector.tensor_add(t[:, qs], t[:, qs], pxh[:, qs])
                        nc.scalar.activation(
                            h[:, qs], t[:, qs], mybir.ActivationFunctionType.Tanh
                        )
                        nc.vector.tensor_mul(h2[:, qs], h[:, qs], oz[:, qs])
                        nc.vector.tensor_add(t[:, qs], h2[:, qs], zs[:, qs])
                        nc.sync.dma_start(
                            out=out.ap()[msl, c * NF + q * cw : c * NF + (q + 1) * cw],
                            in_=t[:, qs],
                        )
                if c == 0 and not with_bias:
                    dma_cols(wx_sb[1], wxt, 1, 0, KD * NF)
                    dma_cols(wh_sb[1], wht, 1, 0, KD * NF)

    nc.compile()
    return nc


def _get_program(with_bias):
    key = ("nc", with_bias)
    if key not in _CACHE:
        _CACHE[key] = _build_program(with_bias)
    return _CACHE[key]


def _retile(w, ngrp):
    """[K, N] -> [ngrp*128 + p, ktile*512 + c] with w[k*128+p, g*512+c] at
    [g*128+p, k*512+c]; one partition's k-span is contiguous."""
    kt = w.shape[0] // P
    return np.ascontiguousarray(
        w.reshape(kt, P, ngrp, NF).transpose(2, 1, 0, 3).reshape(ngrp * P, kt * NF)
    )


def kernel(inp, state, wx, bx, wh, wr, ur, uz, wz, br, bz):
    import ml_dtypes

    bf16 = ml_dtypes.bfloat16
    f8 = ml_dtypes.float8_e4m3fn
    inp = np.asarray(inp, dtype=np.float32)
    state = np.asarray(state, dtype=np.float32)
    w_zr = np.block(
        [
            [np.asarray(wz, np.float32), np.asarray(wr, np.float32)],
            [np.asarray(uz, np.float32), np.asarray(ur, np.float32)],
        ]
    )
    b_zr = np.concatenate(
        [np.asarray(bz, np.float32), np.asarray(br, np.float32)]
    )[None, :]
    b_x = np.asarray(bx, np.float32)[None, :]
    with_bias = bool(np.any(b_zr) or np.any(b_x))

    wx_t = _retile(np.asarray(wx, np.float32).astype(bf16), 2)
    wh_t = _retile(np.asarray(wh, np.float32).astype(bf16), 2)
    common = {"wxt": wx_t, "wht": wh_t}
    if with_bias:
        common["wzrt"] = _retile(w_zr.astype(bf16), 4)
        common["bzr"] = b_zr.astype(bf16)
        common["bx"] = np.ascontiguousarray(b_x.astype(bf16))
    else:
        common["wzr8"] = _retile((w_zr * SW).astype(f8), 4).reshape(
            4 * P, 2 * KD, NF
        )

    in_maps = []
    for cidx in range(N_CORES):
        sl = slice(cidx * BL, (cidx + 1) * BL)
        xT = np.ascontiguousarray(inp[sl].T)  # [D, BL]
        sT = np.ascontiguousarray(state[sl].T)
        im = {
            "xt": _retile(xT.astype(bf16), 2),
            "stt": _retile(sT.astype(bf16), 2),
            "st": np.ascontiguousarray(state[sl]),
            **common,
        }
        if not with_bias:
            im["x8"] = _retile((xT * SX).astype(f8), 2).reshape(2 * P, KD, NF)
            im["s8"] = _retile((sT * SX).astype(f8), 2).reshape(2 * P, KD, NF)
        in_maps.append(im)

    nc = _get_program(with_bias)
    trace = bool(int(os.environ.get("GRU_TRACE", "0")))
    res = run_bass_kernel_spmd(nc, in_maps, list(range(N_CORES)), trace=trace)
    if trace:
        _CACHE["last_exec_time_ns"] = res.exec_time_ns
        _CACHE["last_results"] = res
    return np.concatenate([res.results[c]["out"] for c in range(N_CORES)], axis=0)
